# revision 24
# baseline (speedup 1.0000x reference)
"""Trainium2 Bass kernel (fp8 DoubleRow build) for nn_ErrorCorrectionModule (vq_codebook).

Sharding: head-parallel attention + one ReduceScatter. Core c (batch b=c//4,
rank r=c%4) computes 4 of the 16 heads for ALL 2048 tokens of its batch at
every scale (Q/K/V projections take only the 256 weight columns for those
heads, so per-core FLOPs match the token-parallel split) and multiplies its
heads' context rows into the fused Wo@W1 weight block, yielding a partial
[2048, H] pre-LayerNorm activation. A single ReduceScatter(add) over each
4-core batch group sums the partials and hands each core the 512 tokens it
owns — the collective's output is 1 MB instead of the 15 MB the old
AllGather-K/V scheme moved. The token-local paths (memory-codebook softmax,
error gate, mc@W1, LayerNorm, residual combine) run data-parallel on the
owning core; gate/memory/mc@W1 are issued after the ReduceScatter so they
fill its latency shadow.

On-device layout: activations stay feature-major (features on partitions,
tokens free); attention scores are computed transposed (S^T = K.Q^T per
head, 64-partition contraction, two heads packed per 128 partitions); exp
runs on ScalarE straight out of PSUM; the softmax normalizer falls out of a
ones-augmented V column and is divided out via an fp32r ones-matmul
partition-broadcast. bk drops out exactly (softmax shift invariance); bo@W1
and b1 fold into one host-side bias; the 1/s pooling scale folds into
wq/wk/wv. All matmuls are bf16 with fp32 PSUM accumulation.

This walrus build accepts at most one sync wait per instruction, so a
post-pass (_split_waits) parks Tile's extra waits on standalone
EventSemaphore carriers; the Tile kernel-tail drain gets the same treatment
(_SplitDrainTileContext).
"""
import numpy as np
import ml_dtypes

import concourse.bass as bass
import concourse.tile as tile
import concourse.mybir as mybir
from concourse.bass import ds
from concourse.bass_utils import run_bass_kernel_spmd  # noqa: F401

B, L, H = 2, 2048, 1024
NH, HD = 16, 64
M = 1024
SCALES = (1, 2, 4)
EPS = 1e-6
NCORES = 8
GROUP = 4
TPC = L // GROUP   # 512 own tokens
HPC = NH // GROUP  # 4 own heads

F32 = mybir.dt.float32
BF16 = mybir.dt.bfloat16
AF = mybir.ActivationFunctionType
ALU = mybir.AluOpType
BF = ml_dtypes.bfloat16


class _SplitDrainTileContext(tile.TileContext):
    """Kernel-tail drain must carry <=2 sync waits on this walrus build; park
    the waits on standalone single-wait EventSemaphore instructions."""

    def _drain_and_barrier(self, tick_clock, wait_clock):
        from concourse.vector_clock import ScopedClock
        nc = self.nc
        probe = nc.sync.drain()
        wait_clock.add_sem_waits(
            probe.ins, ScopedClock({None: tick_clock.global_clock}))
        si = probe.ins.sync_info
        waits = list(si.on_wait) if si is not None and si.on_wait else []
        if si is not None and waits:
            si.on_wait = []
            probe.ins.sync_info = si
        sem_by_num = {h.num: h for h in self.sems.allocated().values()}
        for w in waits:
            nc.sync.wait_ge(sem_by_num[w.id], w.wait_value)
        nc.sync.drain()
        nc.all_engine_barrier()
        assert self.sems is not None
        popped = nc._tile_sem_poison_stack.pop()
        assert popped is self._sem_poison
        nc.clear_and_free_semaphores(list(self.sems.allocated().values()))
        nc.all_engine_barrier()


def _split_waits(nc, dma_limit=1, other_limit=1):
    """Walrus on this stack accepts at most 1 sync wait per instruction
    (2 on EventSemaphore); Tile attaches as many as deps require. Move the
    excess onto standalone EventSemaphore carriers inserted just before."""
    ctr = 0
    for f in nc.m.functions:
        for bb in f.blocks:
            out = []
            changed = False
            for inst in bb.instructions:
                si = inst.sync_info
                waits = list(si.on_wait) if si is not None and si.on_wait else []
                tn = type(inst).__name__
                if "EventSemaphore" in tn:
                    limit = 2
                elif "DMA" in tn:
                    limit = dma_limit
                else:
                    limit = other_limit
                if len(waits) > limit:
                    excess = waits[:len(waits) - limit]
                    keep = waits[len(waits) - limit:]
                    for i in range(0, len(excess), 2):
                        ev = mybir.InstEventSemaphore(
                            name=f"WS-{ctr}", ins=[], outs=[])
                        ctr += 1
                        ev.engine = inst.engine
                        ev.sync_info = mybir.SyncInfo(
                            on_wait=excess[i:i + 2], on_update=[])
                        nc.register_instruction(ev, overwrite=True)
                        out.append(ev)
                    si.on_wait = keep
                    inst.sync_info = si
                    changed = True
                out.append(inst)
            if changed:
                bb.instructions = out
    return nc


def _build_program():
    nc = bass.Bass()
    FP8 = mybir.dt.float8e4
    DR = mybir.MatmulPerfMode.DoubleRow

    x8p = nc.declare_dram_parameter("x8p", [H, L], FP8, isOutput=False)
    xo8p = nc.declare_dram_parameter("xo8p", [H, TPC], FP8, isOutput=False)
    xToT = nc.declare_dram_parameter("xToT", [H, TPC], BF16, isOutput=False)
    xt = nc.declare_dram_parameter("xt", [TPC, H], F32, isOutput=False)
    wq = nc.declare_dram_parameter("wq", [3, H, 256], FP8, isOutput=False)
    wk = nc.declare_dram_parameter("wk", [3, H, 256], FP8, isOutput=False)
    wv = nc.declare_dram_parameter("wv", [3, H, 256], FP8, isOutput=False)
    wf = nc.declare_dram_parameter("wf", [3, 256, H], FP8, isOutput=False)
    w1m = nc.declare_dram_parameter("w1m", [H, H], FP8, isOutput=False)
    w2 = nc.declare_dram_parameter("w2", [H, H], BF16, isOutput=False)
    em = nc.declare_dram_parameter("em", [M, H], FP8, isOutput=False)
    emT = nc.declare_dram_parameter("emT", [H, M], FP8, isOutput=False)
    bqh = nc.declare_dram_parameter("bqh", [128, 3, 2], F32, isOutput=False)
    bvv = nc.declare_dram_parameter("bvv", [3, HPC, HD], F32, isOutput=False)
    vecs = nc.declare_dram_parameter("vecs", [4, H], F32, isOutput=False)
    lnv = nc.declare_dram_parameter("lnv", [2, H], BF16, isOutput=False)
    onesr = nc.declare_dram_parameter("onesr", [1, 128],
                                      mybir.dt.float32r, isOutput=False)

    out_y = nc.declare_dram_parameter("out_y", [TPC, H], F32, isOutput=True)
    out_g = nc.declare_dram_parameter("out_g", [TPC, H], F32, isOutput=True)

    def chunked(w):
        return w.rearrange("(c p) n -> p c n", p=128)

    def chunked2(w):
        # DoubleRow k-tile-pair layout: contraction row = c*256 + two*128 + p
        return w.rearrange("(c two p) n -> p c two n", two=2, p=128)

    def rep_vec(row):
        v = vecs[row, :]
        return bass.AP(tensor=v.tensor, offset=v.offset,
                       ap=[[0, 128]] + [list(p) for p in v.ap])

    with _SplitDrainTileContext(nc) as tc:
        with (
            tc.tile_pool(name="const", bufs=1) as const,
            tc.tile_pool(name="persist", bufs=1) as persist,
            tc.tile_pool(name="dram", bufs=1, space="DRAM") as dram,
            tc.tile_pool(name="ppmm", bufs=2, space="PSUM") as ppmm,
            tc.tile_pool(name="ppsp", bufs=2, space="PSUM") as ppsp,
            tc.tile_pool(name="ppctx", bufs=2, space="PSUM") as ppctx,
        ):
            ones2_w = const.tile([128, 2, 128], FP8)
            nc.vector.memset(ones2_w, 1.0)
            ones2_8 = ones2_w[:, :, 0:1]
            ones_r = const.tile([1, 128], mybir.dt.float32r)  # value 64.0
            nc.sync.dma_start(out=ones_r, in_=onesr[:, :])
            epst = const.tile([128, 1], F32)
            nc.vector.memset(epst, EPS)
            bqh_sb = const.tile([128, 3, 2], F32)
            nc.sync.dma_start(out=bqh_sb, in_=bqh[:, :, :])
            bvv_sb = const.tile([128, 3, HPC, HD], F32)
            v_ap = bvv[:, :, :]
            nc.sync.dma_start(out=bvv_sb, in_=bass.AP(
                tensor=v_ap.tensor, offset=v_ap.offset,
                ap=[[0, 128]] + [list(p) for p in v_ap.ap]))

            xOwn = persist.tile([128, 8, TPC], BF16)
            xOwn8 = persist.tile([128, 8, TPC], FP8)
            gate_bf = persist.tile([128, 4, H], BF16)
            sumg = persist.tile([128, 4], F32)
            mcT8 = persist.tile([128, 8, TPC], FP8)
            mcw = persist.tile([128, 4, H], BF16)

            rs_in = dram.tile([L, H], BF16)
            rs_out = dram.tile([TPC, H], BF16)

            with tc.tile_pool(name="kvq", bufs=1) as kvq:
                Ks, Qs, Vs = {}, {}, {}
                for s in SCALES:
                    Tf = L // s
                    Ks[s] = [kvq.tile([64, 2, Tf], FP8,
                                      name=f"K{s}{g}", tag=f"K{s}{g}")
                             for g in range(2)]
                    Qs[s] = [kvq.tile([64, 2, Tf], FP8,
                                      name=f"Q{s}{g}", tag=f"Q{s}{g}")
                             for g in range(2)]
                    Vs[s] = kvq.tile([128, Tf // 128, HPC, 128], FP8,
                                     name=f"V{s}", tag=f"V{s}")

                ctx_memp = tc.tile_pool(name="memp", bufs=1)
                memp = ctx_memp.__enter__()
                ctx_p4w = tc.tile_pool(name="p4w", bufs=1)
                p4w = ctx_p4w.__enter__()
                w2_sb = p4w.tile([128, 8, H], BF16)
                w1m_sb = p4w.tile([128, 4, 2, H], FP8)
                ctx_ctxp = tc.tile_pool(name="ctxp", bufs=1)
                ctxp = ctx_ctxp.__enter__()
                ctxU = {}
                for s in SCALES:
                    ctxU[s] = ctxp.tile([128, 2, L], FP8,
                                        name=f"cU{s}", tag=f"cU{s}")
                em_sb = memp.tile([128, 4, 2, H], FP8)
                emT_sb = memp.tile([128, 4, 2, M], FP8)

                # ---------- P1: Q/K/V projections (fp8 DoubleRow) ----------
                with tc.tile_pool(name="xp", bufs=1) as xp, \
                     tc.tile_pool(name="wstr", bufs=2) as wstr, \
                     tc.tile_pool(name="wfill", bufs=1) as wfill, \
                     tc.tile_pool(name="ctxs", bufs=1) as ctxs:
                    xTo8 = xp.tile([128, 8, L], FP8)
                    nc.sync.dma_start(out=xTo8, in_=chunked(x8p[:, :]))
                    x2o8 = xp.tile([128, 8, L // 2], FP8)
                    x4o8 = xp.tile([128, 8, L // 4], FP8)
                    for c in range(8):
                        h2 = xTo8[:, c, :].rearrange("p (t two) -> p t two",
                                                     two=2)
                        nc.vector.tensor_add(x2o8[:, c, :], h2[:, :, 0],
                                             h2[:, :, 1])
                        h4 = x2o8[:, c, :].rearrange("p (t two) -> p t two",
                                                     two=2)
                        nc.gpsimd.tensor_add(x4o8[:, c, :], h4[:, :, 0],
                                             h4[:, :, 1])
                    xso = {1: xTo8, 2: x2o8, 4: x4o8}

                    def k_block(s, wk_sb, blk, tb, dma=None):
                        xs = xso[s]
                        if dma is not None:
                            nc.sync.dma_start(out=wk_sb, in_=dma)
                        ps = ppmm.tile([128, 512], F32, name="mm", tag="mm")
                        for hp in range(4):
                            nc.tensor.matmul(
                                ps[:, :],
                                wk_sb[:, hp, :, ds(blk * 128, 128)],
                                xs[:, ds(2 * hp, 2), ds(tb * 512, 512)],
                                start=(hp == 0), stop=(hp == 3),
                                perf_mode=DR,
                            )
                        kcp = (nc.scalar.copy if s == 1
                               else nc.vector.tensor_copy)
                        for g in range(2):
                            kcp(out=Ks[s][g][:, blk, ds(tb * 512, 512)],
                                in_=ps[ds(64 * g, 64), :])

                    def q_block(si, s, wq_sb, blk, tb, dma=None):
                        xs = xso[s]
                        if dma is not None:
                            nc.sync.dma_start(out=wq_sb, in_=dma)
                        ps = ppmm.tile([128, 512], F32, name="mm", tag="mm")
                        for hp in range(4):
                            nc.tensor.matmul(
                                ps[:, :],
                                wq_sb[:, hp, :, ds(blk * 128, 128)],
                                xs[:, ds(2 * hp, 2), ds(tb * 512, 512)],
                                start=(hp == 0), stop=(hp == 3),
                                perf_mode=DR,
                            )
                        for g in range(2):
                            nc.vector.tensor_scalar_add(
                                Qs[s][g][:, blk, ds(tb * 512, 512)],
                                ps[ds(64 * g, 64), :],
                                bqh_sb[ds(64 * g, 64), si, ds(blk, 1)])

                    def v_block(si, s, wv_sb, vt, dma=None):
                        xs = xso[s]
                        if dma is not None:
                            nc.sync.dma_start(out=wv_sb, in_=dma)
                            nc.vector.memset(Vs[s][:, :, :, HD:HD + 1], 1.0)
                        ps = ppmm.tile([128, 512], F32, name="mm", tag="mm")
                        for hp in range(4):
                            nc.tensor.matmul(
                                ps[:, 0:256],
                                xs[:, ds(2 * hp, 2), ds(vt * 128, 128)],
                                wv_sb[:, ds(2 * hp, 2), :],
                                start=(hp == 0), stop=(hp == 3),
                                perf_mode=DR,
                            )
                        nc.vector.tensor_add(
                            Vs[s][:, vt, :, 0:HD],
                            ps[:, 0:256].rearrange("p (h d) -> p h d", d=HD),
                            bvv_sb[:, si, :, :])

                    def proj_scale(si, s):
                        Tf = L // s
                        wk_sb = wstr.tile([128, 4, 2, 256], FP8,
                                          name="wkv", tag="wkv")
                        for blk in range(2):
                            for tb in range(Tf // 512):
                                k_block(s, wk_sb, blk, tb,
                                        dma=(chunked2(wk[si, :, :])
                                             if blk == 0 and tb == 0
                                             else None))
                        wq_sb = wstr.tile([128, 4, 2, 256], FP8,
                                          name="wkv", tag="wkv")
                        for blk in range(2):
                            for tb in range(Tf // 512):
                                q_block(si, s, wq_sb, blk, tb,
                                        dma=(chunked2(wq[si, :, :])
                                             if blk == 0 and tb == 0
                                             else None))
                        wv_sb = wstr.tile([128, 8, 256], FP8,
                                          name="wkv", tag="wkv")
                        for vt in range(Tf // 128):
                            v_block(si, s, wv_sb, vt,
                                    dma=(chunked(wv[si, :, :]) if vt == 0
                                         else None))

                    def gen_proj_fillers(si, s, out):
                        Tf = L // s
                        wk_sb = wfill.tile([128, 4, 2, 256], FP8,
                                           name=f"wk{s}", tag=f"wk{s}")
                        wq_sb = wfill.tile([128, 4, 2, 256], FP8,
                                           name=f"wq{s}", tag=f"wq{s}")
                        wv_sb = wfill.tile([128, 8, 256], FP8,
                                           name=f"wv{s}", tag=f"wv{s}")
                        for blk in range(2):
                            for tb in range(Tf // 512):
                                out.append(lambda s=s, w=wk_sb, b=blk, t=tb: (
                                    k_block(s, w, b, t,
                                            dma=(chunked2(wk[si, :, :])
                                                 if b == 0 and t == 0
                                                 else None))))
                        for blk in range(2):
                            for tb in range(Tf // 512):
                                out.append(lambda si=si, s=s, w=wq_sb,
                                           b=blk, t=tb: (
                                    q_block(si, s, w, b, t,
                                            dma=(chunked2(wq[si, :, :])
                                                 if b == 0 and t == 0
                                                 else None))))
                        for vt in range(Tf // 128):
                            out.append(lambda si=si, s=s, w=wv_sb, v=vt: (
                                v_block(si, s, w, v,
                                        dma=(chunked(wv[si, :, :]) if v == 0
                                             else None))))

                    Em8 = memp.tile([128, 8, TPC], FP8, name="Em", tag="Em")
                    zinv_m = memp.tile([1, TPC], mybir.dt.float32r,
                                       name="zinvm", tag="zinvm")
                    zb_m = memp.tile([128, TPC], F32, name="zbm", tag="zbm")
                    b1rep = memp.tile([128, H], F32, name="b1r", tag="b1r")

                    def mem_sim(mt):
                        ps = ppmm.tile([128, 512], F32, name="mm", tag="mm")
                        for hp in range(4):
                            nc.tensor.matmul(
                                ps[:, :],
                                emT_sb[:, hp, :, ds(mt * 128, 128)],
                                xOwn8[:, ds(2 * hp, 2), :],
                                start=(hp == 0), stop=(hp == 3),
                                perf_mode=DR,
                            )
                        nc.scalar.activation(Em8[:, mt, :], ps[:, :],
                                             AF.Exp, scale=1.0 / (32.0 * 16.0))

                    def mem_norm():
                        zps = ppctx.tile([1, TPC], F32, name="ctx", tag="ctx")
                        for i in range(4):
                            nc.tensor.matmul(zps[:, :], ones2_8,
                                             Em8[:, ds(2 * i, 2), :],
                                             start=(i == 0), stop=(i == 3),
                                             perf_mode=DR)
                        with nc.allow_low_precision(reason="softmax bcast"):
                            nc.vector.reciprocal(out=zinv_m, in_=zps[:, :])
                        zbp = ppmm.tile([128, TPC], F32, name="mm", tag="mm")
                        nc.tensor.matmul(zbp[:, :], ones_r[:, :],
                                         zinv_m[:, :], start=True, stop=True)
                        nc.vector.tensor_copy(out=zb_m, in_=zbp[:, :])

                    def mem_read(ht):
                        ps = ppmm.tile([128, 512], F32, name="mm", tag="mm")
                        for mp in range(4):
                            nc.tensor.matmul(
                                ps[:, :],
                                em_sb[:, mp, :, ds(ht * 128, 128)],
                                Em8[:, ds(2 * mp, 2), :],
                                start=(mp == 0), stop=(mp == 3),
                                perf_mode=DR,
                            )
                        nc.vector.tensor_mul(mcT8[:, ht, :], ps[:, :],
                                             zb_m[:, :])

                    def mem_w1(tt, jh, dma=False):
                        if dma:
                            nc.sync.dma_start(out=b1rep, in_=rep_vec(0))
                        ps = ppmm.tile([128, 512], F32, name="mm", tag="mm")
                        for hp in range(4):
                            nc.tensor.matmul(
                                ps[:, :],
                                mcT8[:, ds(2 * hp, 2), ds(tt * 128, 128)],
                                w1m_sb[:, hp, :, ds(jh * 512, 512)],
                                start=(hp == 0), stop=(hp == 3),
                                perf_mode=DR,
                            )
                        nc.vector.scalar_tensor_tensor(
                            out=mcw[:, tt, ds(jh * 512, 512)],
                            in0=ps[:, :], scalar=1.0 / 16384.0,
                            in1=b1rep[:, ds(jh * 512, 512)],
                            op0=ALU.mult, op1=ALU.add,
                        )

                    def gen_mem_fillers(out):
                        for mt in range(8):
                            out.append(lambda mt=mt: mem_sim(mt))
                        out.append(mem_norm)
                        for ht in range(8):
                            out.append(lambda ht=ht: mem_read(ht))
                        for tt in range(4):
                            for jh in range(2):
                                out.append(lambda tt=tt, jh=jh: (
                                    mem_w1(tt, jh,
                                           dma=(tt == 0 and jh == 0))))

                    # -------- emit: s1 proj, then attn with fillers --------
                    proj_scale(0, 1)

                    nc.sync.dma_start(out=xOwn8, in_=chunked(xo8p[:, :]))
                    nc.sync.dma_start(out=em_sb, in_=chunked2(em[:, :]))
                    nc.sync.dma_start(out=emT_sb, in_=chunked2(emT[:, :]))
                    nc.sync.dma_start(out=w2_sb, in_=chunked(w2[:, :]))
                    nc.sync.dma_start(out=w1m_sb, in_=chunked2(w1m[:, :]))
                    nc.sync.dma_start(out=xOwn, in_=chunked(xToT[:, :]))

                    fillers = []
                    gen_proj_fillers(1, 2, fillers)
                    gen_proj_fillers(2, 4, fillers)
                    gen_mem_fillers(fillers)

                    _attn_scale(nc, tc, 1, Ks[1], Qs[1], Vs[1], bvv_sb,
                                ctxU[1], ppsp, ppctx, ppmm, ones_r,
                                fillers=fillers)
                    for s in (2, 4):
                        Tf = L // s
                        ctxT = ctxs.tile([128, 2, Tf], FP8,
                                         name=f"cT{s}", tag=f"cT{s}")
                        _attn_scale(nc, tc, s, Ks[s], Qs[s], Vs[s], bvv_sb,
                                    ctxT, ppsp, ppctx, ppmm, ones_r,
                                    fillers=fillers)
                        for pair in range(2):
                            nc.gpsimd.tensor_copy(
                                out=ctxU[s][:, pair, :].rearrange(
                                    "p (t r) -> p t r", r=s),
                                in_=ctxT[:, pair, :].unsqueeze(-1)
                                .broadcast_to([128, Tf, s]))
                    while fillers:
                        fillers.pop(0)()

                # ---------- P3: fused Wo@W1 partial + ReduceScatter --------
                with tc.tile_pool(name="wfp", bufs=1) as wfp, \
                     tc.tile_pool(name="rsw", bufs=3) as rsw:
                    wf_sb = wfp.tile([128, 3, 2, H], FP8)
                    for i in range(3):
                        nc.sync.dma_start(out=wf_sb[:, i, :, :],
                                          in_=chunked(wf[i, :, :]))
                    for tt in range(L // 128):
                        cfs = rsw.tile([128, H], BF16, name="cfs", tag="cfs")
                        for jh in range(2):
                            ps = ppmm.tile([128, 512], F32,
                                           name="mm", tag="mm")
                            for si, s in enumerate(SCALES):
                                nc.tensor.matmul(
                                    ps[:, :],
                                    ctxU[s][:, :, ds(tt * 128, 128)],
                                    wf_sb[:, si, :, ds(jh * 512, 512)],
                                    start=(si == 0), stop=(si == 2),
                                    perf_mode=DR,
                                )
                            nc.scalar.copy(
                                out=cfs[:, ds(jh * 512, 512)], in_=ps[:, :])
                        nc.sync.dma_start(out=rs_in[ds(tt * 128, 128), :],
                                          in_=cfs)
                    nc.gpsimd.collective_compute(
                        "ReduceScatter", mybir.AluOpType.add,
                        replica_groups=[[0, 1, 2, 3], [4, 5, 6, 7]],
                        ins=[rs_in.opt()], outs=[rs_out.opt()])
                ctx_ctxp.__exit__(None, None, None)

                # ---------- P4: error gate (RS shadow) ----------
                with tc.tile_pool(name="gw", bufs=2) as gw:
                    b2rep = memp.tile([128, H], F32, name="b2r", tag="b2r")
                    nc.sync.dma_start(out=b2rep, in_=rep_vec(1))
                    for tt in range(4):
                        gf = gw.tile([128, H], F32, name="gatef", tag="gatef")
                        for jh in range(2):
                            ps = ppmm.tile([128, 512], F32,
                                           name="mm", tag="mm")
                            for hc in range(8):
                                nc.tensor.matmul(
                                    ps[:, :],
                                    xOwn[:, hc, ds(tt * 128, 128)],
                                    w2_sb[:, hc, ds(jh * 512, 512)],
                                    start=(hc == 0), stop=(hc == 7),
                                )
                            lg = gw.tile([128, 512], F32, name="lg", tag="lg")
                            nc.vector.tensor_add(lg, ps[:, :],
                                                 b2rep[:, ds(jh * 512, 512)])
                            th = gw.tile([128, 512], F32, name="th", tag="th")
                            nc.scalar.activation(th, lg, AF.Tanh, scale=0.5)
                            nc.vector.tensor_scalar(
                                out=gf[:, ds(jh * 512, 512)], in0=th,
                                scalar1=0.5, scalar2=0.5,
                                op0=ALU.mult, op1=ALU.add,
                            )
                        nc.gpsimd.tensor_copy(out=gate_bf[:, tt, :], in_=gf)
                        nc.vector.reduce_sum(out=sumg[:, ds(tt, 1)], in_=gf,
                                             axis=mybir.AxisListType.X)
                        nc.sync.dma_start(out=out_g[ds(tt * 128, 128), :],
                                          in_=gf)
                ctx_p4w.__exit__(None, None, None)
                ctx_memp.__exit__(None, None, None)

            # ---------- P5: post-RS tail ----------
            with tc.tile_pool(name="fwork", bufs=2) as fwork, \
                 tc.tile_pool(name="finp", bufs=1) as finp:
                rs2 = finp.tile([128, 4, H], BF16)
                nc.sync.dma_start(
                    out=rs2, in_=rs_out.rearrange("(t p) f -> p t f", p=128))
                def rep_ln(row):
                    v = lnv[row, :]
                    return bass.AP(tensor=v.tensor, offset=v.offset,
                                   ap=[[0, 128]] + [list(p) for p in v.ap])
                lnsrep = finp.tile([128, H], BF16)
                nc.sync.dma_start(out=lnsrep, in_=rep_ln(0))
                lnbrep = finp.tile([128, H], BF16)
                nc.sync.dma_start(out=lnbrep, in_=rep_ln(1))
                for tt in range(4):
                    cfr = fwork.tile([128, H], F32, name="cfr", tag="cfr")
                    nc.vector.scalar_tensor_tensor(
                        out=cfr, in0=rs2[:, tt, :], scalar=1.0 / 1024.0,
                        in1=mcw[:, tt, :], op0=ALU.mult, op1=ALU.add,
                    )
                    stats = fwork.tile([128, 2, 6], F32,
                                       name="stats", tag="stats")
                    for jh in range(2):
                        nc.vector.bn_stats(out=stats[:, jh, :],
                                           in_=cfr[:, ds(jh * 512, 512)])
                    mv = fwork.tile([128, 2], F32, name="mv", tag="mv")
                    nc.vector.bn_aggr(out=mv, in_=stats)
                    rstd = fwork.tile([128, 1], F32, name="rstd", tag="rstd")
                    nc.scalar.activation(rstd, mv[:, 1:2], AF.Sqrt,
                                         bias=epst[:, :])
                    nc.vector.reciprocal(out=rstd, in_=rstd)
                    nb = fwork.tile([128, 1], F32, name="nb", tag="nb")
                    nc.vector.scalar_tensor_tensor(
                        out=nb, in0=mv[:, 0:1], scalar=-1.0, in1=rstd,
                        op0=ALU.mult, op1=ALU.mult,
                    )
                    cf = fwork.tile([128, H], BF16, name="cf", tag="cf")
                    nc.vector.tensor_scalar(
                        out=cf, in0=cfr, scalar1=rstd[:, :], scalar2=nb[:, :],
                        op0=ALU.mult, op1=ALU.add,
                    )
                    nc.vector.tensor_mul(cf, cf, lnsrep)
                    nc.vector.tensor_add(cf, cf, lnbrep)
                    cfx = fwork.tile([128, H], BF16, name="cfx", tag="cfx")
                    nc.scalar.activation(cfx, cf, AF.Relu)

                    sc = fwork.tile([128, 1], F32, name="sc", tag="sc")
                    nc.scalar.activation(sc, sumg[:, ds(tt, 1)], AF.Sqrt,
                                         bias=epst[:, :])
                    nc.vector.reciprocal(out=sc, in_=sc)
                    xin = fwork.tile([128, H], F32, name="xin", tag="xin")
                    nc.sync.dma_start(out=xin, in_=xt[ds(tt * 128, 128), :])
                    gc = fwork.tile([128, H], BF16, name="gc", tag="gc")
                    nc.vector.tensor_mul(gc, gate_bf[:, tt, :], cfx)
                    yout = fwork.tile([128, H], F32, name="yout", tag="yout")
                    nc.vector.scalar_tensor_tensor(
                        out=yout, in0=gc, scalar=sc[:, :], in1=xin,
                        op0=ALU.mult, op1=ALU.add,
                    )
                    nc.sync.dma_start(out=out_y[ds(tt * 128, 128), :],
                                      in_=yout)

    return _split_waits(nc)


def _attn_scale(nc, tc, s, K_sb, Q_sb, V_sb, _bv_unused, ctxT_s,
                ppsp, ppctx, ppmm, ones_r, fillers=None):
    DR = mybir.MatmulPerfMode.DoubleRow
    FP8 = mybir.dt.float8e4
    Tf = L // s
    nkt = Tf // 128
    nfill = 0
    with tc.tile_pool(name=f"awork{s}", bufs=3) as awork, \
         tc.tile_pool(name=f"azb{s}", bufs=2) as azb:
        for h in range(HPC):
            pair = h // 2
            po = (h % 2) * 64
            for qb in range(Tf // 512):
                ctx = ppctx.tile([65, 512], F32, name="ctx", tag="ctx")
                for kp in range(nkt // 2):
                    sp = ppsp.tile([128, 1024], F32, name="sp", tag="sp")
                    for half in range(2):
                        kt = kp * 2 + half
                        nc.tensor.matmul(
                            sp[:, ds(half * 512, 512)],
                            K_sb[h // 2][ds(32 * (h % 2), 32), :,
                                         ds(kt * 128, 128)],
                            Q_sb[h // 2][ds(32 * (h % 2), 32), :,
                                         ds(qb * 512, 512)],
                            start=True, stop=True,
                            perf_mode=DR,
                        )
                    e8 = awork.tile([128, 2, 512], FP8, name="esb", tag="esb")
                    nc.scalar.activation(e8[:, :, :], sp[:, :], AF.Exp,
                                         scale=0.125)
                    nc.tensor.matmul(
                        ctx[:, :],
                        V_sb[:, ds(2 * kp, 2), h, 0:HD + 1],
                        e8[:, :, :],
                        start=(kp == 0), stop=(kp == nkt // 2 - 1),
                        perf_mode=DR,
                    )
                    nfill += 1
                    if fillers and nfill % 2 == 0:
                        fillers.pop(0)()
                zinv = awork.tile([1, 512], mybir.dt.float32r,
                                  name="zinva", tag="zinva")
                with nc.allow_low_precision(reason="softmax norm bcast"):
                    nc.vector.reciprocal(out=zinv, in_=ctx[64:65, :])
                zbp = ppmm.tile([64, 512], F32, name="mm", tag="mm")
                nc.tensor.matmul(zbp[:, :], ones_r[:, 0:64], zinv[:, :],
                                 start=True, stop=True)
                zb = azb.tile([64, 512], F32, name="zba", tag="zba")
                nc.vector.tensor_copy(out=zb, in_=zbp[:, :])
                nc.vector.tensor_mul(
                    ctxT_s[ds(po, 64), pair, ds(qb * 512, 512)],
                    ctx[0:64, :], zb)


_CACHE = {}


def _get_program():
    if "nc" not in _CACHE:
        _CACHE["nc"] = _build_program()
    return _CACHE["nc"]


def _prep_host(inputs):
    x = np.asarray(inputs["x"], np.float32)
    emx = np.asarray(inputs["error_memory"], np.float32)
    Wq = np.asarray(inputs["Wq"], np.float32)
    Wk = np.asarray(inputs["Wk"], np.float32)
    Wv = np.asarray(inputs["Wv"], np.float32)
    Wo = np.asarray(inputs["Wo"], np.float32)
    W1 = np.asarray(inputs["W1"], np.float32)
    W2 = np.asarray(inputs["W2"], np.float32)
    bq = np.asarray(inputs["bq"], np.float32)
    bv = np.asarray(inputs["bv"], np.float32)
    bo = np.asarray(inputs["bo"], np.float32)
    b1 = np.asarray(inputs["b1"], np.float32)
    b2 = np.asarray(inputs["b2"], np.float32)
    lns = np.asarray(inputs["ln_scale"], np.float32)
    lnb = np.asarray(inputs["ln_bias"], np.float32)

    E4 = ml_dtypes.float8_e4m3

    scl = np.array([1.0, 0.5, 0.25], np.float32)
    wq_h = Wq * scl[:, None, None]
    wk_h = Wk * scl[:, None, None]
    wv_h = Wv * scl[:, None, None]
    wf_h = np.stack([Wo[i] @ W1[i * H:(i + 1) * H] for i in range(3)])
    w1m_h = (W1[3 * H:4 * H] * 16.0).astype(E4)
    w2_h = W2.astype(BF)
    em_h = (emx * 16.0).astype(E4)
    emT_h = np.ascontiguousarray(emx.T * 16.0).astype(E4)
    b1e = b1 + sum(bo[i] @ W1[i * H:(i + 1) * H] for i in range(3))
    vecs_h = np.stack([b1e, b2, lns, lnb]).astype(np.float32)
    lnv_h = np.stack([lns, lnb]).astype(BF)

    # score-layout permutation: within a core's 256 cols, block i (0=lo,1=hi)
    # holds [h0 d(32i..32i+32) | h1 ... | h2 | h3]
    perm = np.array([h * 64 + i * 32 + p
                     for i in range(2) for h in range(4) for p in range(32)])

    shared = dict(w1m=w1m_h, w2=w2_h, em=em_h, emT=emT_h, vecs=vecs_h,
                  lnv=lnv_h,
                  onesr=np.full((1, 128), 64.0, np.float32))

    in_maps = []
    for c in range(NCORES):
        b, r = divmod(c, GROUP)
        cols = slice(r * 256, (r + 1) * 256)
        own = x[b, r * TPC:(r + 1) * TPC]
        bqh_h = np.stack([bq[i, r * 256:(r + 1) * 256][perm].reshape(2, 128)
                          for i in range(3)])             # [3, 2, 128]
        bvv_h = np.stack([
            [bv[i, (4 * r + h) * 64:(4 * r + h + 1) * 64]
             for h in range(HPC)] for i in range(3)])       # [3, 4, 64]
        m = dict(shared)
        m["x8p"] = np.ascontiguousarray(x[b].T).astype(E4)
        m["xo8p"] = np.ascontiguousarray(own.T).astype(E4)
        m["xToT"] = np.ascontiguousarray(own.T).astype(BF)
        m["xt"] = np.ascontiguousarray(own).astype(np.float32)
        m["wq"] = np.ascontiguousarray(
            wq_h[:, :, cols][:, :, perm]).astype(E4)
        m["wk"] = np.ascontiguousarray(
            wk_h[:, :, cols][:, :, perm]).astype(E4)
        m["wv"] = np.ascontiguousarray(wv_h[:, :, cols]).astype(E4)
        m["wf"] = np.ascontiguousarray(wf_h[:, cols, :] * 16.0).astype(E4)
        m["bqh"] = np.ascontiguousarray(
            bqh_h.transpose(2, 0, 1)).astype(np.float32)  # [128, 3, 2]
        m["bvv"] = np.ascontiguousarray(bvv_h).astype(np.float32)
        in_maps.append(m)
    return in_maps


def _runner():
    """Build (once) a cached jitted 8-core executable for this program."""
    if "run" in _CACHE:
        return _CACHE["run"]
    import jax
    from jax.experimental.shard_map import shard_map
    from jax.sharding import Mesh, PartitionSpec
    from concourse import bass2jax

    nc = _get_program()
    bass2jax.install_neuronx_cc_hook()
    partition_name = (nc.partition_id_tensor.name
                      if nc.partition_id_tensor else None)
    in_names, out_names, out_avals = [], [], []
    for alloc in nc.m.functions[0].allocations:
        if not isinstance(alloc, mybir.MemoryLocationSet):
            continue
        name = alloc.memorylocations[0].name
        if alloc.kind == "ExternalInput":
            if name != partition_name:
                in_names.append(name)
        elif alloc.kind == "ExternalOutput":
            out_names.append(name)
            out_avals.append(jax.core.ShapedArray(
                tuple(alloc.tensor_shape), mybir.dt.np(alloc.dtype)))
    n_params = len(in_names)
    n_outs = len(out_avals)
    all_in = list(in_names) + list(out_names)
    if partition_name is not None:
        all_in.append(partition_name)
    donate = tuple(range(n_params, n_params + n_outs))

    def _body(*args):
        operands = list(args)
        if partition_name is not None:
            operands.append(bass2jax.partition_id_tensor())
        outs = bass2jax._bass_exec_p.bind(
            *operands,
            out_avals=tuple(out_avals),
            in_names=tuple(all_in),
            out_names=tuple(out_names),
            lowering_input_output_aliases=(),
            sim_require_finite=True,
            sim_require_nnan=True,
            nc=nc,
        )
        return tuple(outs)

    devices = jax.devices()[:NCORES]
    mesh = Mesh(np.asarray(devices), ("core",))
    in_specs = (PartitionSpec("core"),) * (n_params + n_outs)
    out_specs = (PartitionSpec("core"),) * n_outs
    fn = jax.jit(
        shard_map(_body, mesh=mesh, in_specs=in_specs,
                  out_specs=out_specs, check_rep=False),
        donate_argnums=donate, keep_unused=True)
    _CACHE["run"] = (fn, in_names, out_names, out_avals, mesh)
    return _CACHE["run"]


def _concat_inputs(in_maps, in_names):
    return [np.concatenate([np.asarray(in_maps[c][n]) for c in range(NCORES)],
                           axis=0) for n in in_names]


def kernel(**inputs):
    fn, in_names, out_names, out_avals, mesh = _runner()
    in_maps = _prep_host(inputs)
    concat_in = _concat_inputs(in_maps, in_names)
    zeros = [np.zeros((NCORES * a.shape[0], *a.shape[1:]), a.dtype)
             for a in out_avals]
    outs = fn(*concat_in, *zeros)
    res = {n: np.asarray(outs[i]) for i, n in enumerate(out_names)}
    y = res["out_y"].reshape(B, L, H)
    g = res["out_g"].reshape(B, L, H)
    return y, g


# revision 39
# speedup vs baseline: 1.0692x; 1.0692x over previous
"""Trainium2 Bass kernel (fp8 DoubleRow build) for nn_ErrorCorrectionModule (vq_codebook).

Sharding: head-parallel attention + one ReduceScatter. Core c (batch b=c//4,
rank r=c%4) computes 4 of the 16 heads for ALL 2048 tokens of its batch at
every scale (Q/K/V projections take only the 256 weight columns for those
heads, so per-core FLOPs match the token-parallel split) and multiplies its
heads' context rows into the fused Wo@W1 weight block, yielding a partial
[2048, H] pre-LayerNorm activation. A single ReduceScatter(add) over each
4-core batch group sums the partials and hands each core the 512 tokens it
owns — the collective's output is 1 MB instead of the 15 MB the old
AllGather-K/V scheme moved. The token-local paths (memory-codebook softmax,
error gate, mc@W1, LayerNorm, residual combine) run data-parallel on the
owning core; gate/memory/mc@W1 are issued after the ReduceScatter so they
fill its latency shadow.

On-device layout: activations stay feature-major (features on partitions,
tokens free); attention scores are computed transposed (S^T = K.Q^T per
head, 64-partition contraction, two heads packed per 128 partitions); exp
runs on ScalarE straight out of PSUM; the softmax normalizer falls out of a
ones-augmented V column and is divided out via an fp32r ones-matmul
partition-broadcast. bk drops out exactly (softmax shift invariance); bo@W1
and b1 fold into one host-side bias; the 1/s pooling scale folds into
wq/wk/wv. All matmuls are bf16 with fp32 PSUM accumulation.

This walrus build accepts at most one sync wait per instruction, so a
post-pass (_split_waits) parks Tile's extra waits on standalone
EventSemaphore carriers; the Tile kernel-tail drain gets the same treatment
(_SplitDrainTileContext).
"""
import numpy as np
import ml_dtypes

import concourse.bass as bass
import concourse.tile as tile
import concourse.mybir as mybir
from concourse.bass import ds
from concourse.bass_utils import run_bass_kernel_spmd  # noqa: F401

B, L, H = 2, 2048, 1024
NH, HD = 16, 64
M = 1024
SCALES = (1, 2, 4)
EPS = 1e-6
NCORES = 8
GROUP = 4
TPC = L // GROUP   # 512 own tokens
HPC = NH // GROUP  # 4 own heads

F32 = mybir.dt.float32
BF16 = mybir.dt.bfloat16
AF = mybir.ActivationFunctionType
ALU = mybir.AluOpType
BF = ml_dtypes.bfloat16


class _SplitDrainTileContext(tile.TileContext):
    """Kernel-tail drain must carry <=2 sync waits on this walrus build; park
    the waits on standalone single-wait EventSemaphore instructions."""

    def _drain_and_barrier(self, tick_clock, wait_clock):
        from concourse.vector_clock import ScopedClock
        nc = self.nc
        probe = nc.sync.drain()
        wait_clock.add_sem_waits(
            probe.ins, ScopedClock({None: tick_clock.global_clock}))
        si = probe.ins.sync_info
        waits = list(si.on_wait) if si is not None and si.on_wait else []
        if si is not None and waits:
            si.on_wait = []
            probe.ins.sync_info = si
        sem_by_num = {h.num: h for h in self.sems.allocated().values()}
        for w in waits:
            nc.sync.wait_ge(sem_by_num[w.id], w.wait_value)
        nc.sync.drain()
        nc.all_engine_barrier()
        assert self.sems is not None
        popped = nc._tile_sem_poison_stack.pop()
        assert popped is self._sem_poison
        nc.clear_and_free_semaphores(list(self.sems.allocated().values()))
        nc.all_engine_barrier()


def _split_waits(nc, dma_limit=1, other_limit=1):
    """Walrus on this stack accepts at most 1 sync wait per instruction
    (2 on EventSemaphore); Tile attaches as many as deps require. Move the
    excess onto standalone EventSemaphore carriers inserted just before."""
    ctr = 0
    for f in nc.m.functions:
        for bb in f.blocks:
            out = []
            changed = False
            for inst in bb.instructions:
                si = inst.sync_info
                waits = list(si.on_wait) if si is not None and si.on_wait else []
                tn = type(inst).__name__
                if "EventSemaphore" in tn:
                    limit = 2
                elif "DMA" in tn:
                    limit = dma_limit
                else:
                    limit = other_limit
                if len(waits) > limit:
                    excess = waits[:len(waits) - limit]
                    keep = waits[len(waits) - limit:]
                    for i in range(0, len(excess), 2):
                        ev = mybir.InstEventSemaphore(
                            name=f"WS-{ctr}", ins=[], outs=[])
                        ctr += 1
                        ev.engine = inst.engine
                        ev.sync_info = mybir.SyncInfo(
                            on_wait=excess[i:i + 2], on_update=[])
                        nc.register_instruction(ev, overwrite=True)
                        out.append(ev)
                    si.on_wait = keep
                    inst.sync_info = si
                    changed = True
                out.append(inst)
            if changed:
                bb.instructions = out
    return nc


def _build_program(flags=frozenset()):
    has_bq = "bq" in flags
    has_b2 = "b2" in flags
    has_ln = "ln" in flags
    nc = bass.Bass()
    FP8 = mybir.dt.float8e4
    DR = mybir.MatmulPerfMode.DoubleRow

    x8p = nc.declare_dram_parameter("x8p", [H, L], FP8, isOutput=False)
    xo8p = nc.declare_dram_parameter("xo8p", [H, TPC], FP8, isOutput=False)
    xToT = nc.declare_dram_parameter("xToT", [H, TPC], BF16, isOutput=False)
    xt = nc.declare_dram_parameter("xt", [TPC, H], F32, isOutput=False)
    wq = nc.declare_dram_parameter("wq", [3, H, 256], FP8, isOutput=False)
    wk = nc.declare_dram_parameter("wk", [3, H, 256], FP8, isOutput=False)
    wv = nc.declare_dram_parameter("wv", [3, H, 256], FP8, isOutput=False)
    wf = nc.declare_dram_parameter("wf", [3, 256, H], FP8, isOutput=False)
    w1m = nc.declare_dram_parameter("w1m", [H, H], FP8, isOutput=False)
    w2 = nc.declare_dram_parameter("w2", [H, H], BF16, isOutput=False)
    em = nc.declare_dram_parameter("em", [M, H], FP8, isOutput=False)
    emT = nc.declare_dram_parameter("emT", [H, M], FP8, isOutput=False)
    bqh = nc.declare_dram_parameter("bqh", [128, 3, 2], F32, isOutput=False)
    bvv = nc.declare_dram_parameter("bvv", [3, HPC, HD], F32, isOutput=False)
    vecs = nc.declare_dram_parameter("vecs", [4, H], F32, isOutput=False)
    lnv = nc.declare_dram_parameter("lnv", [2, H], BF16, isOutput=False)
    onesr = nc.declare_dram_parameter("onesr", [1, 128],
                                      mybir.dt.float32r, isOutput=False)

    out_y = nc.declare_dram_parameter("out_y", [TPC, H], F32, isOutput=True)
    out_g = nc.declare_dram_parameter("out_g", [TPC, H], F32, isOutput=True)

    def chunked(w):
        return w.rearrange("(c p) n -> p c n", p=128)

    def chunked2(w):
        # DoubleRow k-tile-pair layout: contraction row = c*256 + two*128 + p
        return w.rearrange("(c two p) n -> p c two n", two=2, p=128)

    def rep_vec(row):
        v = vecs[row, :]
        return bass.AP(tensor=v.tensor, offset=v.offset,
                       ap=[[0, 128]] + [list(p) for p in v.ap])

    with _SplitDrainTileContext(nc) as tc:
        with (
            tc.tile_pool(name="const", bufs=1) as const,
            tc.tile_pool(name="persist", bufs=1) as persist,
            tc.tile_pool(name="dram", bufs=1, space="DRAM") as dram,
            tc.tile_pool(name="ppmm", bufs=2, space="PSUM") as ppmm,
            tc.tile_pool(name="ppsp", bufs=2, space="PSUM") as ppsp,
            tc.tile_pool(name="ppctx", bufs=2, space="PSUM") as ppctx,
        ):
            ones2_w = const.tile([128, 2, 128], FP8)
            nc.vector.memset(ones2_w, 1.0)
            ones2_8 = ones2_w[:, :, 0:1]
            ones_r = const.tile([1, 128], mybir.dt.float32r)  # value 64.0
            nc.sync.dma_start(out=ones_r, in_=onesr[:, :])
            epst = const.tile([128, 1], F32)
            nc.vector.memset(epst, EPS)
            bqh_sb = const.tile([128, 3, 2], F32)
            nc.sync.dma_start(out=bqh_sb, in_=bqh[:, :, :])
            bvv_sb = const.tile([128, 3, HPC, HD], F32)
            v_ap = bvv[:, :, :]
            nc.sync.dma_start(out=bvv_sb, in_=bass.AP(
                tensor=v_ap.tensor, offset=v_ap.offset,
                ap=[[0, 128]] + [list(p) for p in v_ap.ap]))

            xOwn = persist.tile([128, 8, TPC], BF16)
            xOwn8 = persist.tile([128, 8, TPC], FP8)
            gate_bf = persist.tile([128, 4, H], BF16)
            sumg = persist.tile([128, 4], F32)
            mcT8 = persist.tile([128, 8, TPC], FP8)
            mcw = persist.tile([128, 4, H], BF16)

            rs_in = dram.tile([L, H], FP8)
            rs_out = dram.tile([TPC, H], FP8)

            with tc.tile_pool(name="kvq", bufs=1) as kvq:
                Ks, Qs, Vs = {}, {}, {}
                for s in SCALES:
                    Tf = L // s
                    Ks[s] = [kvq.tile([64, 2, Tf], FP8,
                                      name=f"K{s}{g}", tag=f"K{s}{g}")
                             for g in range(2)]
                    Qs[s] = [kvq.tile([64, 2, Tf], FP8,
                                      name=f"Q{s}{g}", tag=f"Q{s}{g}")
                             for g in range(2)]
                    Vs[s] = kvq.tile([128, Tf // 128, HPC, 96], FP8,
                                     name=f"V{s}", tag=f"V{s}")

                ctx_memp = tc.tile_pool(name="memp", bufs=1)
                memp = ctx_memp.__enter__()
                ctx_p4w = tc.tile_pool(name="p4w", bufs=1)
                p4w = ctx_p4w.__enter__()
                w2_sb = p4w.tile([128, 8, H], BF16)
                w1m_sb = p4w.tile([128, 4, 2, H], FP8)
                wf_sb = p4w.tile([128, 3, 2, H], FP8)
                ctx_rsw = tc.tile_pool(name="rsw", bufs=3)
                rsw = ctx_rsw.__enter__()
                ctx_ctxp = tc.tile_pool(name="ctxp", bufs=1)
                ctxp = ctx_ctxp.__enter__()
                ctxU = {}
                for s in SCALES:
                    ctxU[s] = ctxp.tile([128, 2, L], FP8,
                                        name=f"cU{s}", tag=f"cU{s}")
                em_sb = memp.tile([128, 4, 2, H], FP8)
                emT_sb = memp.tile([128, 4, 2, M], FP8)

                # ---------- P1: Q/K/V projections (fp8 DoubleRow) ----------
                with tc.tile_pool(name="xp", bufs=1) as xp, \
                     tc.tile_pool(name="wstr", bufs=2) as wstr, \
                     tc.tile_pool(name="wfill", bufs=1) as wfill, \
                     tc.tile_pool(name="ctxs", bufs=1) as ctxs:
                    xTo8 = xp.tile([128, 8, L], FP8)
                    x2o8 = xp.tile([128, 8, L // 2], FP8)
                    x4o8 = xp.tile([128, 8, L // 4], FP8)
                    for half in range(2):
                        for tb4 in range(2 * half, 2 * half + 2):
                            nc.sync.dma_start(
                                out=xTo8[:, :, ds(tb4 * 512, 512)],
                                in_=chunked(x8p[:, :])[:, :,
                                                       ds(tb4 * 512, 512)])
                        for c in range(8):
                            h2 = xTo8[:, c, ds(half * 1024, 1024)].rearrange(
                                "p (t two) -> p t two", two=2)
                            nc.vector.tensor_add(
                                x2o8[:, c, ds(half * 512, 512)],
                                h2[:, :, 0], h2[:, :, 1])
                    for c in range(8):
                        h4 = x2o8[:, c, :].rearrange("p (t two) -> p t two",
                                                     two=2)
                        nc.gpsimd.tensor_add(x4o8[:, c, :], h4[:, :, 0],
                                             h4[:, :, 1])
                    xso = {1: xTo8, 2: x2o8, 4: x4o8}

                    def k_block(s, wk_sb, blk, tb, dma=None, act=False):
                        xs = xso[s]
                        if dma is not None:
                            nc.sync.dma_start(out=wk_sb, in_=dma)
                        ps = ppmm.tile([128, 512], F32, name="mm", tag="mm")
                        for hp in range(4):
                            nc.tensor.matmul(
                                ps[:, :],
                                wk_sb[:, hp, :, ds(blk * 128, 128)],
                                xs[:, ds(2 * hp, 2), ds(tb * 512, 512)],
                                start=(hp == 0), stop=(hp == 3),
                                perf_mode=DR,
                            )
                        kcp = nc.scalar.copy if act else nc.vector.tensor_copy
                        for g in range(2):
                            kcp(out=Ks[s][g][:, blk, ds(tb * 512, 512)],
                                in_=ps[ds(64 * g, 64), :])

                    def q_block(si, s, wq_sb, blk, tb, dma=None, act=False):
                        xs = xso[s]
                        if dma is not None:
                            nc.sync.dma_start(out=wq_sb, in_=dma)
                        ps = ppmm.tile([128, 512], F32, name="mm", tag="mm")
                        for hp in range(4):
                            nc.tensor.matmul(
                                ps[:, :],
                                wq_sb[:, hp, :, ds(blk * 128, 128)],
                                xs[:, ds(2 * hp, 2), ds(tb * 512, 512)],
                                start=(hp == 0), stop=(hp == 3),
                                perf_mode=DR,
                            )
                        for g in range(2):
                            if has_bq:
                                nc.vector.tensor_scalar_add(
                                    Qs[s][g][:, blk, ds(tb * 512, 512)],
                                    ps[ds(64 * g, 64), :],
                                    bqh_sb[ds(64 * g, 64), si, ds(blk, 1)])
                            elif act:
                                nc.scalar.copy(
                                    out=Qs[s][g][:, blk, ds(tb * 512, 512)],
                                    in_=ps[ds(64 * g, 64), :])
                            else:
                                nc.vector.tensor_copy(
                                    out=Qs[s][g][:, blk, ds(tb * 512, 512)],
                                    in_=ps[ds(64 * g, 64), :])

                    def v_block(si, s, wv_sb, vt, dma=None):
                        xs = xso[s]
                        if dma is not None:
                            nc.sync.dma_start(out=wv_sb, in_=dma)
                            nc.vector.memset(Vs[s][:, :, :, HD:HD + 1], 1.0)
                        ps = ppmm.tile([128, 512], F32, name="mm", tag="mm")
                        for hp in range(4):
                            nc.tensor.matmul(
                                ps[:, 0:256],
                                xs[:, ds(2 * hp, 2), ds(vt * 128, 128)],
                                wv_sb[:, ds(2 * hp, 2), :],
                                start=(hp == 0), stop=(hp == 3),
                                perf_mode=DR,
                            )
                        nc.vector.tensor_add(
                            Vs[s][:, vt, :, 0:HD],
                            ps[:, 0:256].rearrange("p (h d) -> p h d", d=HD),
                            bvv_sb[:, si, :, :])

                    def proj_scale(si, s):
                        Tf = L // s
                        wk_sb = wstr.tile([128, 4, 2, 256], FP8,
                                          name="wkv", tag="wkv")
                        for blk in range(2):
                            for tb in range(Tf // 512):
                                k_block(s, wk_sb, blk, tb,
                                        dma=(chunked2(wk[si, :, :])
                                             if blk == 0 and tb == 0
                                             else None))
                        wq_sb = wstr.tile([128, 4, 2, 256], FP8,
                                          name="wkv", tag="wkv")
                        for blk in range(2):
                            for tb in range(Tf // 512):
                                q_block(si, s, wq_sb, blk, tb,
                                        dma=(chunked2(wq[si, :, :])
                                             if blk == 0 and tb == 0
                                             else None))
                        wv_sb = wstr.tile([128, 8, 256], FP8,
                                          name="wkv", tag="wkv")
                        for vt in range(Tf // 128):
                            v_block(si, s, wv_sb, vt,
                                    dma=(chunked(wv[si, :, :]) if vt == 0
                                         else None))

                    def gen_proj_fillers(si, s, out):
                        Tf = L // s
                        wk_sb = wfill.tile([128, 4, 2, 256], FP8,
                                           name=f"wk{s}", tag=f"wk{s}")
                        wq_sb = wfill.tile([128, 4, 2, 256], FP8,
                                           name=f"wq{s}", tag=f"wq{s}")
                        wv_sb = wfill.tile([128, 8, 256], FP8,
                                           name=f"wv{s}", tag=f"wv{s}")
                        for blk in range(2):
                            for tb in range(Tf // 512):
                                out.append(lambda s=s, w=wk_sb, b=blk, t=tb: (
                                    k_block(s, w, b, t,
                                            dma=(chunked2(wk[si, :, :])
                                                 if b == 0 and t == 0
                                                 else None))))
                        for blk in range(2):
                            for tb in range(Tf // 512):
                                out.append(lambda si=si, s=s, w=wq_sb,
                                           b=blk, t=tb: (
                                    q_block(si, s, w, b, t,
                                            dma=(chunked2(wq[si, :, :])
                                                 if b == 0 and t == 0
                                                 else None))))
                        for vt in range(Tf // 128):
                            out.append(lambda si=si, s=s, w=wv_sb, v=vt: (
                                v_block(si, s, w, v,
                                        dma=(chunked(wv[si, :, :]) if v == 0
                                             else None))))

                    Em8 = memp.tile([128, 8, TPC], FP8, name="Em", tag="Em")
                    zinv_m = memp.tile([1, TPC], mybir.dt.float32r,
                                       name="zinvm", tag="zinvm")
                    zb_m = memp.tile([128, TPC], F32, name="zbm", tag="zbm")
                    b1rep = memp.tile([128, H], F32, name="b1r", tag="b1r")

                    def mem_sim(mt):
                        ps = ppmm.tile([128, 512], F32, name="mm", tag="mm")
                        for hp in range(4):
                            nc.tensor.matmul(
                                ps[:, :],
                                emT_sb[:, hp, :, ds(mt * 128, 128)],
                                xOwn8[:, ds(2 * hp, 2), :],
                                start=(hp == 0), stop=(hp == 3),
                                perf_mode=DR,
                            )
                        nc.scalar.activation(Em8[:, mt, :], ps[:, :],
                                             AF.Exp, scale=1.0 / (32.0 * 16.0))

                    def mem_norm():
                        zps = ppctx.tile([1, TPC], F32, name="ctx", tag="ctx")
                        for i in range(4):
                            nc.tensor.matmul(zps[:, :], ones2_8,
                                             Em8[:, ds(2 * i, 2), :],
                                             start=(i == 0), stop=(i == 3),
                                             perf_mode=DR)
                        with nc.allow_low_precision(reason="softmax bcast"):
                            nc.vector.reciprocal(out=zinv_m, in_=zps[:, :])
                        zbp = ppmm.tile([128, TPC], F32, name="mm", tag="mm")
                        nc.tensor.matmul(zbp[:, :], ones_r[:, :],
                                         zinv_m[:, :], start=True, stop=True)
                        nc.vector.tensor_copy(out=zb_m, in_=zbp[:, :])

                    def mem_read(ht):
                        ps = ppmm.tile([128, 512], F32, name="mm", tag="mm")
                        for mp in range(4):
                            nc.tensor.matmul(
                                ps[:, :],
                                em_sb[:, mp, :, ds(ht * 128, 128)],
                                Em8[:, ds(2 * mp, 2), :],
                                start=(mp == 0), stop=(mp == 3),
                                perf_mode=DR,
                            )
                        nc.vector.tensor_mul(mcT8[:, ht, :], ps[:, :],
                                             zb_m[:, :])

                    def mem_w1(tt, jh, dma=False):
                        if dma:
                            nc.sync.dma_start(out=b1rep, in_=rep_vec(0))
                        ps = ppmm.tile([128, 512], F32, name="mm", tag="mm")
                        for hp in range(4):
                            nc.tensor.matmul(
                                ps[:, :],
                                mcT8[:, ds(2 * hp, 2), ds(tt * 128, 128)],
                                w1m_sb[:, hp, :, ds(jh * 512, 512)],
                                start=(hp == 0), stop=(hp == 3),
                                perf_mode=DR,
                            )
                        nc.vector.scalar_tensor_tensor(
                            out=mcw[:, tt, ds(jh * 512, 512)],
                            in0=ps[:, :], scalar=1.0 / 16384.0,
                            in1=b1rep[:, ds(jh * 512, 512)],
                            op0=ALU.mult, op1=ALU.add,
                        )

                    def gen_mem_fillers(out):
                        for mt in range(8):
                            out.append(lambda mt=mt: mem_sim(mt))
                        out.append(mem_norm)
                        for ht in range(8):
                            out.append(lambda ht=ht: mem_read(ht))
                        for tt in range(4):
                            for jh in range(2):
                                out.append(lambda tt=tt, jh=jh: (
                                    mem_w1(tt, jh,
                                           dma=(tt == 0 and jh == 0))))

                    # -------- emit: minimal s2 prefix (tb0 K/Q + all V) ----
                    wk2 = wfill.tile([128, 4, 2, 256], FP8,
                                     name="wk2u", tag="wk4")
                    wq2 = wfill.tile([128, 4, 2, 256], FP8,
                                     name="wq2u", tag="wq4")
                    wv2 = wfill.tile([128, 8, 256], FP8,
                                     name="wv2u", tag="wv4")
                    k_block(2, wk2, 0, 0, dma=chunked2(wk[1, :, :]),
                            act=True)
                    k_block(2, wk2, 1, 0, act=True)
                    nc.sync.dma_start(out=wv2, in_=chunked(wv[1, :, :]))
                    nc.vector.memset(Vs[2][:, :, :, HD:HD + 1], 1.0)
                    q_block(1, 2, wq2, 0, 0, dma=chunked2(wq[1, :, :]),
                            act=True)
                    q_block(1, 2, wq2, 1, 0, act=True)
                    for vt in range(8):
                        v_block(1, 2, wv2, vt)

                    nc.sync.dma_start(out=xOwn8, in_=chunked(xo8p[:, :]))
                    nc.sync.dma_start(out=em_sb, in_=chunked2(em[:, :]))
                    nc.sync.dma_start(out=emT_sb, in_=chunked2(emT[:, :]))
                    nc.sync.dma_start(out=w2_sb, in_=chunked(w2[:, :]))
                    nc.sync.dma_start(out=w1m_sb, in_=chunked2(w1m[:, :]))
                    nc.sync.dma_start(out=xOwn, in_=chunked(xToT[:, :]))
                    for i in range(3):
                        nc.sync.dma_start(out=wf_sb[:, i, :, :],
                                          in_=chunked(wf[i, :, :]))

                    proj_fillers = [
                        lambda: k_block(2, wk2, 0, 1),
                        lambda: k_block(2, wk2, 1, 1),
                        lambda: q_block(1, 2, wq2, 0, 1),
                        lambda: q_block(1, 2, wq2, 1, 1),
                    ]
                    gen_proj_fillers(2, 4, proj_fillers)
                    gen_proj_fillers(0, 1, proj_fillers)
                    mem_fillers = []
                    gen_mem_fillers(mem_fillers)

                    def wf_tt(tt):
                        cfs = rsw.tile([128, H], FP8, name="cfs", tag="cfs")
                        for jh in range(2):
                            ps = ppmm.tile([128, 512], F32,
                                           name="mm", tag="mm")
                            for si, s in enumerate(SCALES):
                                nc.tensor.matmul(
                                    ps[:, :],
                                    ctxU[s][:, :, ds(tt * 128, 128)],
                                    wf_sb[:, si, :, ds(jh * 512, 512)],
                                    start=(si == 0), stop=(si == 2),
                                    perf_mode=DR,
                                )
                            nc.vector.tensor_copy(
                                out=cfs[:, ds(jh * 512, 512)], in_=ps[:, :])
                        nc.sync.dma_start(out=rs_in[ds(tt * 128, 128), :],
                                          in_=cfs)

                    def s1_hook(qb):
                        for tt in range(4 * qb, 4 * qb + 4):
                            wf_tt(tt)

                    for s in (2, 4):
                        Tf = L // s
                        ctxT = ctxs.tile([128, 2, Tf], FP8,
                                         name=f"cT{s}", tag=f"cT{s}")
                        _attn_scale(nc, tc, s, Ks[s], Qs[s], Vs[s], bvv_sb,
                                    ctxT, ppsp, ppctx, ppmm, ones_r,
                                    fillers=proj_fillers, fill_every=1)
                        if s == 2:
                            while len(proj_fillers) > 36:
                                proj_fillers.pop(0)()
                        for pair in range(2):
                            nc.gpsimd.tensor_copy(
                                out=ctxU[s][:, pair, :].rearrange(
                                    "p (t r) -> p t r", r=s),
                                in_=ctxT[:, pair, :].unsqueeze(-1)
                                .broadcast_to([128, Tf, s]))

                    while proj_fillers:
                        proj_fillers.pop(0)()
                    _attn_scale(nc, tc, 1, Ks[1], Qs[1], Vs[1], bvv_sb,
                                ctxU[1], ppsp, ppctx, ppmm, ones_r,
                                fillers=mem_fillers, fill_every=2,
                                qb_hook=s1_hook)
                    while mem_fillers:
                        mem_fillers.pop(0)()

                # ---------- P3: ReduceScatter ----------
                nc.gpsimd.collective_compute(
                    "ReduceScatter", mybir.AluOpType.add,
                    replica_groups=[[0, 1, 2, 3], [4, 5, 6, 7]],
                    ins=[rs_in.opt()], outs=[rs_out.opt()])
                ctx_ctxp.__exit__(None, None, None)

                # ---------- P4: error gate (RS shadow) ----------
                with tc.tile_pool(name="gw", bufs=2) as gw:
                    if has_b2:
                        b2rep = memp.tile([128, H], F32,
                                          name="b2r", tag="b2r")
                        nc.sync.dma_start(out=b2rep, in_=rep_vec(1))
                    for tt in range(4):
                        gf = gw.tile([128, H], F32, name="gatef", tag="gatef")
                        for jh in range(2):
                            ps = ppmm.tile([128, 512], F32,
                                           name="mm", tag="mm")
                            for hc in range(8):
                                nc.tensor.matmul(
                                    ps[:, :],
                                    xOwn[:, hc, ds(tt * 128, 128)],
                                    w2_sb[:, hc, ds(jh * 512, 512)],
                                    start=(hc == 0), stop=(hc == 7),
                                )
                            th = gw.tile([128, 512], F32, name="th", tag="th")
                            if has_b2:
                                lg = gw.tile([128, 512], F32,
                                             name="lg", tag="lg")
                                nc.vector.tensor_add(
                                    lg, ps[:, :], b2rep[:, ds(jh * 512, 512)])
                                nc.scalar.activation(th, lg, AF.Tanh,
                                                     scale=0.5)
                            else:
                                nc.scalar.activation(th, ps[:, :], AF.Tanh,
                                                     scale=0.5)
                            nc.vector.tensor_scalar(
                                out=gf[:, ds(jh * 512, 512)], in0=th,
                                scalar1=0.5, scalar2=0.5,
                                op0=ALU.mult, op1=ALU.add,
                            )
                        nc.gpsimd.tensor_copy(out=gate_bf[:, tt, :], in_=gf)
                        nc.vector.reduce_sum(out=sumg[:, ds(tt, 1)], in_=gf,
                                             axis=mybir.AxisListType.X)
                        nc.sync.dma_start(out=out_g[ds(tt * 128, 128), :],
                                          in_=gf)
                ctx_rsw.__exit__(None, None, None)
                ctx_p4w.__exit__(None, None, None)
                ctx_memp.__exit__(None, None, None)

            # ---------- P5: post-RS tail ----------
            with tc.tile_pool(name="fwork", bufs=2) as fwork, \
                 tc.tile_pool(name="finp", bufs=1) as finp:
                rs2 = finp.tile([128, 4, H], FP8)
                xin4 = finp.tile([128, 4, H], F32)
                nc.sync.dma_start(
                    out=xin4, in_=xt.rearrange("(t p) f -> p t f", p=128))
                nc.sync.dma_start(
                    out=rs2, in_=rs_out.rearrange("(t p) f -> p t f", p=128))
                def rep_ln(row):
                    v = lnv[row, :]
                    return bass.AP(tensor=v.tensor, offset=v.offset,
                                   ap=[[0, 128]] + [list(p) for p in v.ap])
                if has_ln:
                    lnsrep = finp.tile([128, H], BF16)
                    nc.sync.dma_start(out=lnsrep, in_=rep_ln(0))
                    lnbrep = finp.tile([128, H], BF16)
                    nc.sync.dma_start(out=lnbrep, in_=rep_ln(1))
                for tt in range(4):
                    cfr = fwork.tile([128, H], F32, name="cfr", tag="cfr")
                    nc.vector.scalar_tensor_tensor(
                        out=cfr, in0=rs2[:, tt, :], scalar=1.0 / 1024.0,
                        in1=mcw[:, tt, :], op0=ALU.mult, op1=ALU.add,
                    )
                    stats = fwork.tile([128, 2, 6], F32,
                                       name="stats", tag="stats")
                    for jh in range(2):
                        nc.vector.bn_stats(out=stats[:, jh, :],
                                           in_=cfr[:, ds(jh * 512, 512)])
                    mv = fwork.tile([128, 2], F32, name="mv", tag="mv")
                    nc.vector.bn_aggr(out=mv, in_=stats)
                    rstd = fwork.tile([128, 1], F32, name="rstd", tag="rstd")
                    nc.scalar.activation(rstd, mv[:, 1:2], AF.Sqrt,
                                         bias=epst[:, :])
                    nc.vector.reciprocal(out=rstd, in_=rstd)
                    nb = fwork.tile([128, 1], F32, name="nb", tag="nb")
                    nc.vector.scalar_tensor_tensor(
                        out=nb, in0=mv[:, 0:1], scalar=-1.0, in1=rstd,
                        op0=ALU.mult, op1=ALU.mult,
                    )
                    cf = fwork.tile([128, H], BF16, name="cf", tag="cf")
                    nc.vector.tensor_scalar(
                        out=cf, in0=cfr, scalar1=rstd[:, :], scalar2=nb[:, :],
                        op0=ALU.mult, op1=ALU.add,
                    )
                    if has_ln:
                        nc.vector.tensor_mul(cf, cf, lnsrep)
                        nc.vector.tensor_add(cf, cf, lnbrep)
                    cfx = fwork.tile([128, H], BF16, name="cfx", tag="cfx")
                    nc.scalar.activation(cfx, cf, AF.Relu)

                    sc = fwork.tile([128, 1], F32, name="sc", tag="sc")
                    nc.scalar.activation(sc, sumg[:, ds(tt, 1)], AF.Sqrt,
                                         bias=epst[:, :])
                    nc.vector.reciprocal(out=sc, in_=sc)
                    gc = fwork.tile([128, H], BF16, name="gc", tag="gc")
                    nc.vector.tensor_mul(gc, gate_bf[:, tt, :], cfx)
                    yout = fwork.tile([128, H], F32, name="yout", tag="yout")
                    nc.vector.scalar_tensor_tensor(
                        out=yout, in0=gc, scalar=sc[:, :], in1=xin4[:, tt, :],
                        op0=ALU.mult, op1=ALU.add,
                    )
                    nc.sync.dma_start(out=out_y[ds(tt * 128, 128), :],
                                      in_=yout)

    return _split_waits(nc)


def _attn_scale(nc, tc, s, K_sb, Q_sb, V_sb, _bv_unused, ctxT_s,
                ppsp, ppctx, ppmm, ones_r, fillers=None, fill_every=2,
                qb_hook=None):
    """Transposed-scores attention for one scale, qb-major so qb_hook(qb)
    fires when a 512-query block is complete across all 4 heads."""
    DR = mybir.MatmulPerfMode.DoubleRow
    FP8 = mybir.dt.float8e4
    Tf = L // s
    nkt = Tf // 128
    nfill = 0
    with tc.tile_pool(name=f"awork{s}", bufs=3) as awork, \
         tc.tile_pool(name=f"azb{s}", bufs=2) as azb:
        for qb in range(Tf // 512):
            for h in range(HPC):
                pair = h // 2
                po = (h % 2) * 64
                ctx = ppctx.tile([65, 512], F32, name="ctx", tag="ctx")

                def ctx_mm(kp, e8):
                    nc.tensor.matmul(
                        ctx[:, :],
                        V_sb[:, ds(2 * kp, 2), h, 0:HD + 1],
                        e8[:, :, :],
                        start=(kp == 0), stop=(kp == nkt // 2 - 1),
                        perf_mode=DR,
                    )

                pend = None
                for kp in range(nkt // 2):
                    sp = ppsp.tile([128, 1024], F32, name="sp", tag="sp")
                    for half in range(2):
                        kt = kp * 2 + half
                        nc.tensor.matmul(
                            sp[:, ds(half * 512, 512)],
                            K_sb[h // 2][ds(32 * (h % 2), 32), :,
                                         ds(kt * 128, 128)],
                            Q_sb[h // 2][ds(32 * (h % 2), 32), :,
                                         ds(qb * 512, 512)],
                            start=True, stop=True,
                            perf_mode=DR,
                        )
                    e8 = awork.tile([128, 2, 512], FP8, name="esb", tag="esb")
                    nc.scalar.activation(e8[:, :, :], sp[:, :], AF.Exp,
                                         scale=0.125)
                    if pend is not None:
                        ctx_mm(*pend)
                    pend = (kp, e8)
                    nfill += 1
                    if fillers and nfill % fill_every == 0:
                        fillers.pop(0)()
                ctx_mm(*pend)
                zinv = awork.tile([1, 512], mybir.dt.float32r,
                                  name="zinva", tag="zinva")
                with nc.allow_low_precision(reason="softmax norm bcast"):
                    nc.vector.reciprocal(out=zinv, in_=ctx[64:65, :])
                zbp = ppmm.tile([64, 512], F32, name="mm", tag="mm")
                nc.tensor.matmul(zbp[:, :], ones_r[:, 0:64], zinv[:, :],
                                 start=True, stop=True)
                zb = azb.tile([64, 512], F32, name="zba", tag="zba")
                nc.vector.tensor_copy(out=zb, in_=zbp[:, :])
                nc.vector.tensor_mul(
                    ctxT_s[ds(po, 64), pair, ds(qb * 512, 512)],
                    ctx[0:64, :], zb)
            if qb_hook is not None:
                qb_hook(qb)


_CACHE = {}


def _get_program(flags=frozenset()):
    key = ("nc", tuple(sorted(flags)))
    if key not in _CACHE:
        _CACHE[key] = _build_program(flags)
    return _CACHE[key]


def _prep_host(inputs):
    x = np.asarray(inputs["x"], np.float32)
    emx = np.asarray(inputs["error_memory"], np.float32)
    Wq = np.asarray(inputs["Wq"], np.float32)
    Wk = np.asarray(inputs["Wk"], np.float32)
    Wv = np.asarray(inputs["Wv"], np.float32)
    Wo = np.asarray(inputs["Wo"], np.float32)
    W1 = np.asarray(inputs["W1"], np.float32)
    W2 = np.asarray(inputs["W2"], np.float32)
    bq = np.asarray(inputs["bq"], np.float32)
    bv = np.asarray(inputs["bv"], np.float32)
    bo = np.asarray(inputs["bo"], np.float32)
    b1 = np.asarray(inputs["b1"], np.float32)
    b2 = np.asarray(inputs["b2"], np.float32)
    lns = np.asarray(inputs["ln_scale"], np.float32)
    lnb = np.asarray(inputs["ln_bias"], np.float32)

    E4 = ml_dtypes.float8_e4m3

    scl = np.array([1.0, 0.5, 0.25], np.float32)
    wq_h = Wq * scl[:, None, None]
    wk_h = Wk * scl[:, None, None]
    wv_h = Wv * scl[:, None, None]
    wf_h = np.stack([Wo[i] @ W1[i * H:(i + 1) * H] for i in range(3)])
    w1m_h = (W1[3 * H:4 * H] * 16.0).astype(E4)
    w2_h = W2.astype(BF)
    em_h = (emx * 16.0).astype(E4)
    emT_h = np.ascontiguousarray(emx.T * 16.0).astype(E4)
    b1e = b1 + sum(bo[i] @ W1[i * H:(i + 1) * H] for i in range(3))
    vecs_h = np.stack([b1e, b2, lns, lnb]).astype(np.float32)
    lnv_h = np.stack([lns, lnb]).astype(BF)

    flags = set()
    if np.any(bq != 0):
        flags.add("bq")
    if np.any(b2 != 0):
        flags.add("b2")
    if np.any(lns != 1) or np.any(lnb != 0):
        flags.add("ln")

    # score-layout permutation: within a core's 256 cols, block i (0=lo,1=hi)
    # holds [h0 d(32i..32i+32) | h1 ... | h2 | h3]
    perm = np.array([h * 64 + i * 32 + p
                     for i in range(2) for h in range(4) for p in range(32)])

    shared = dict(w1m=w1m_h, w2=w2_h, em=em_h, emT=emT_h, vecs=vecs_h,
                  lnv=lnv_h,
                  onesr=np.full((1, 128), 64.0, np.float32))

    in_maps = []
    for c in range(NCORES):
        b, r = divmod(c, GROUP)
        cols = slice(r * 256, (r + 1) * 256)
        own = x[b, r * TPC:(r + 1) * TPC]
        bqh_h = np.stack([bq[i, r * 256:(r + 1) * 256][perm].reshape(2, 128)
                          for i in range(3)])             # [3, 2, 128]
        bvv_h = np.stack([
            [bv[i, (4 * r + h) * 64:(4 * r + h + 1) * 64]
             for h in range(HPC)] for i in range(3)])       # [3, 4, 64]
        m = dict(shared)
        m["x8p"] = np.ascontiguousarray(x[b].T).astype(E4)
        m["xo8p"] = np.ascontiguousarray(own.T).astype(E4)
        m["xToT"] = np.ascontiguousarray(own.T).astype(BF)
        m["xt"] = np.ascontiguousarray(own).astype(np.float32)
        m["wq"] = np.ascontiguousarray(
            wq_h[:, :, cols][:, :, perm]).astype(E4)
        m["wk"] = np.ascontiguousarray(
            wk_h[:, :, cols][:, :, perm]).astype(E4)
        m["wv"] = np.ascontiguousarray(wv_h[:, :, cols]).astype(E4)
        m["wf"] = np.ascontiguousarray(wf_h[:, cols, :] * 16.0).astype(E4)
        m["bqh"] = np.ascontiguousarray(
            bqh_h.transpose(2, 0, 1)).astype(np.float32)  # [128, 3, 2]
        m["bvv"] = np.ascontiguousarray(bvv_h).astype(np.float32)
        in_maps.append(m)
    return in_maps, frozenset(flags)


def _runner(flags=frozenset()):
    """Build (once) a cached jitted 8-core executable for this program."""
    key = ("run", tuple(sorted(flags)))
    if key in _CACHE:
        return _CACHE[key]
    import jax
    from jax.experimental.shard_map import shard_map
    from jax.sharding import Mesh, PartitionSpec
    from concourse import bass2jax

    nc = _get_program(flags)
    bass2jax.install_neuronx_cc_hook()
    partition_name = (nc.partition_id_tensor.name
                      if nc.partition_id_tensor else None)
    in_names, out_names, out_avals = [], [], []
    for alloc in nc.m.functions[0].allocations:
        if not isinstance(alloc, mybir.MemoryLocationSet):
            continue
        name = alloc.memorylocations[0].name
        if alloc.kind == "ExternalInput":
            if name != partition_name:
                in_names.append(name)
        elif alloc.kind == "ExternalOutput":
            out_names.append(name)
            out_avals.append(jax.core.ShapedArray(
                tuple(alloc.tensor_shape), mybir.dt.np(alloc.dtype)))
    n_params = len(in_names)
    n_outs = len(out_avals)
    all_in = list(in_names) + list(out_names)
    if partition_name is not None:
        all_in.append(partition_name)
    donate = tuple(range(n_params, n_params + n_outs))

    def _body(*args):
        operands = list(args)
        if partition_name is not None:
            operands.append(bass2jax.partition_id_tensor())
        outs = bass2jax._bass_exec_p.bind(
            *operands,
            out_avals=tuple(out_avals),
            in_names=tuple(all_in),
            out_names=tuple(out_names),
            lowering_input_output_aliases=(),
            sim_require_finite=True,
            sim_require_nnan=True,
            nc=nc,
        )
        return tuple(outs)

    devices = jax.devices()[:NCORES]
    mesh = Mesh(np.asarray(devices), ("core",))
    in_specs = (PartitionSpec("core"),) * (n_params + n_outs)
    out_specs = (PartitionSpec("core"),) * n_outs
    fn = jax.jit(
        shard_map(_body, mesh=mesh, in_specs=in_specs,
                  out_specs=out_specs, check_rep=False),
        donate_argnums=donate, keep_unused=True)
    _CACHE[key] = (fn, in_names, out_names, out_avals, mesh)
    return _CACHE[key]


def _concat_inputs(in_maps, in_names):
    return [np.concatenate([np.asarray(in_maps[c][n]) for c in range(NCORES)],
                           axis=0) for n in in_names]


def kernel(**inputs):
    in_maps, flags = _prep_host(inputs)
    fn, in_names, out_names, out_avals, mesh = _runner(flags)
    concat_in = _concat_inputs(in_maps, in_names)
    zeros = [np.zeros((NCORES * a.shape[0], *a.shape[1:]), a.dtype)
             for a in out_avals]
    outs = fn(*concat_in, *zeros)
    res = {n: np.asarray(outs[i]) for i, n in enumerate(out_names)}
    y = res["out_y"].reshape(B, L, H)
    g = res["out_g"].reshape(B, L, H)
    return y, g
